# revision 1
# baseline (speedup 1.0000x reference)
"""Multi-head attention (B=2, C=64, H=W=64, nh=8) on 8 TRN2 NeuronCores.

Sharding: 16 (batch, head) pairs -> 2 consecutive heads per core.
core = 4*b + p handles batch b, heads {2p, 2p+1} = conv channels [16p, 16p+16).

Per-core pipeline (all on-chip, scores never hit HBM):
  conv1x1 (PE fp32r, bias via ones-row)
  -> DRAM-bounce gathers to build Q^T/K^T [8,4096] (fp32r) and V-chunked
     [128,32,9] (bf16, ones column for the softmax denominator)
     (torch .view semantics: Q[n,d] = conv[ch0 + n//512, (n%512)*8 + d])
  -> S^T tiles [128m, 512n] = K Q^T on PE (fp32r, 11-bit mantissa), exp on ACT
     with 1/sqrt(8) folded into the activation scale, bf16 output
     (no max subtraction; scores are O(1) by construction)
  -> PV via augmented V|ones bf16 matmul -> [9, 512] psum (row 8 = denom)
  -> normalize: broadcast denom via PE ones outer product, DVE reciprocal,
     fused gamma*pv*recip (scalar_tensor_tensor), add residual.
"""

import numpy as np

import concourse.bacc as bacc
import concourse.bass as bass
import concourse.tile as tile
from concourse import mybir
from concourse.bass_utils import run_bass_kernel_spmd

F32 = mybir.dt.float32
F32R = mybir.dt.float32r
BF16 = mybir.dt.bfloat16

B = 2
C = 64
N = 4096          # H*W
NH = 8
HD = 8            # head dim
HPC = 2           # heads per core
CH = HPC * HD     # 16 conv channels per core
NCORES = 8
NBLK = N // 512   # 8 n-blocks of 512 query positions
MCHUNK = N // 128  # 32 m-chunks of 128 key positions
EXPW = 1536       # elements exp'd per ACT instruction (psum banks = EXPW/512)
ST_BUFS = 2       # st tile double buffering
ACC_BUFS = 2      # PV accumulator buffering
RB_TAG = "acc"    # pool tag for the broadcast psum tile ("st" or "acc")
SCALE = 1.0 / np.sqrt(float(HD))


def _chunk_groups():
    """Partition the 32 m-chunks into groups of <= EXPW//512 for one exp each."""
    per = EXPW // 512
    groups, k = [], 0
    while k < MCHUNK:
        n = min(per, MCHUNK - k)
        groups.append(list(range(k, k + n)))
        k += n
    return groups


def _emit(tc, xb_d, xres_d, wcat_d, gamma8_d, out_d, scr):
    nc = tc.nc

    with (
        tc.tile_pool(name="persist", bufs=1) as per,
        tc.tile_pool(name="ptp", bufs=3) as ptp,
        tc.tile_pool(name="epl", bufs=3) as epl,
        tc.tile_pool(name="stp", bufs=ST_BUFS, space="PSUM") as stp,
        tc.tile_pool(name="accp", bufs=ACC_BUFS, space="PSUM") as accp,
    ):
        # ---- persistent tiles ----
        g8col = per.tile([HD, 1], F32)            # gamma per-partition scalar
        ones8r = per.tile([1, HD], F32R)          # broadcast lhsT
        xrs = [per.tile([HD, N], F32, name=f"xrs{h}", tag=f"xrs{h}") for h in range(HPC)]
        qt = [per.tile([HD, N], F32R, name=f"qt{h}", tag=f"qt{h}") for h in range(HPC)]
        kt = [per.tile([HD, N], F32R, name=f"kt{h}", tag=f"kt{h}") for h in range(HPC)]
        vc = [per.tile([128, MCHUNK, 33], BF16, name=f"vc{h}", tag=f"vc{h}")
              for h in range(HPC)]

        # ---- loads + fp32r casts ----
        nc.sync.dma_start(out=g8col[:], in_=gamma8_d[:])
        for h in range(HPC):
            nc.sync.dma_start(out=xrs[h][:], in_=xres_d[h * HD : (h + 1) * HD, :])

        o8f = per.tile([1, HD], F32)
        nc.vector.memset(o8f, 1.0)
        nc.vector.tensor_copy(ones8r, o8f)        # rounds to fp32r

        with tc.tile_pool(name="convin", bufs=1) as cin:
            xba = cin.tile([C + 1, N], F32)       # x[b] + ones row (bias)
            xbar = cin.tile([C + 1, N], F32R)
            nc.vector.memset(xba[C : C + 1, :], 1.0)
            nc.sync.dma_start(out=xba[0:C, :], in_=xb_d[:])
            nc.vector.tensor_copy(xbar, xba)      # rounds to fp32r

            wcf = cin.tile([C + 1, 3 * CH], F32)
            nc.sync.dma_start(out=wcf[:], in_=wcat_d[:])
            wcr = cin.tile([C + 1, 3 * CH], F32R)
            nc.vector.tensor_copy(wcr, wcf)       # rounds to fp32r

            # ---- conv1x1: [16,4096] = wT.T @ [65,4096] per q/k/v ----
            with tc.tile_pool(name="convout", bufs=1) as cop:
                cqkv = [
                    cop.tile([CH, N], F32R if t < 2 else BF16, name=f"c{t}", tag=f"c{t}")
                    for t in range(3)
                ]
                for t in range(3):
                    lhsT = wcr[:, t * CH : (t + 1) * CH]
                    for j in range(NBLK):
                        ps = stp.tile([128, EXPW], F32, tag="st")
                        nc.tensor.matmul(
                            ps[0:CH, 0:512],
                            lhsT=lhsT,
                            rhs=xbar[:, j * 512 : (j + 1) * 512],
                            start=True,
                            stop=True,
                        )
                        nc.vector.tensor_copy(
                            cqkv[t][:, j * 512 : (j + 1) * 512], ps[0:CH, 0:512]
                        )
                    nc.sync.dma_start(out=scr[t][:], in_=cqkv[t][:])


                # ---- re-layout gathers (torch .view semantics) ----
                for h in range(HPC):
                    rows = slice(h * HD, (h + 1) * HD)
                    # Q^T/K^T [d, n]: element = conv[row r, col 8t+d], n = 512r+t
                    for r0 in range(0, HD, 2):
                        nc.sync.dma_start(
                            out=qt[h][:].rearrange("d (r t) -> d r t", r=HD)[
                                :, r0 : r0 + 2, :
                            ],
                            in_=scr[0][rows, :].rearrange("r (t d) -> d r t", d=HD)[
                                :, r0 : r0 + 2, :
                            ],
                        )
                        nc.sync.dma_start(
                            out=kt[h][:].rearrange("d (r t) -> d r t", r=HD)[
                                :, r0 : r0 + 2, :
                            ],
                            in_=scr[1][rows, :].rearrange("r (t d) -> d r t", d=HD)[
                                :, r0 : r0 + 2, :
                            ],
                        )
                    # V chunked [i, chunk, d]: m = 128*chunk + i, chunk = 4r+tb
                    nc.sync.dma_start(
                        out=vc[h][:, :, 0:HD],
                        in_=scr[2][rows, :].rearrange(
                            "r (tb i d) -> i (r tb) d", tb=4, i=128, d=HD
                        ),
                    )
                    nc.vector.memset(vc[h][:, :, HD:32], 0.0)
                    nc.vector.memset(vc[h][:, :, 32:33], 1.0)

        # ---- attention per head / n-block ----
        for h in range(HPC):
            och = slice(h * HD, (h + 1) * HD)
            for j in range(NBLK):
                qblk = qt[h][:, j * 512 : (j + 1) * 512]
                acc = accp.tile([33, 512], F32, tag="acc")
                for grp in _chunk_groups():
                    st = stp.tile([128, EXPW], F32, tag="st")
                    for u, k in enumerate(grp):
                        nc.tensor.matmul(
                            st[:, u * 512 : (u + 1) * 512],
                            lhsT=kt[h][:, k * 128 : (k + 1) * 128],
                            rhs=qblk,
                            start=True,
                            stop=True,
                        )
                    w = len(grp) * 512
                    pt = ptp.tile([128, EXPW], BF16)
                    nc.scalar.activation(
                        pt[:, 0:w], st[:, 0:w],
                        mybir.ActivationFunctionType.Exp, scale=SCALE
                    )
                    for u, k in enumerate(grp):
                        nc.tensor.matmul(
                            acc[:, :],
                            lhsT=vc[h][:, k, :],
                            rhs=pt[:, u * 512 : (u + 1) * 512],
                            start=(k == 0),
                            stop=(k == MCHUNK - 1),
                        )

                # ---- epilogue: out = gamma * pv / denom + residual ----
                sb = epl.tile([1, 512], F32R, tag="sb")
                nc.vector.tensor_copy(sb, acc[32:33, :])  # denom -> fp32r
                if RB_TAG == "acc":
                    rb = accp.tile([33, 512], F32, tag="acc")
                else:
                    rbt = stp.tile([128, EXPW], F32, name="rbt", tag="st")
                    rb = rbt[0:33, 0:512]
                nc.tensor.matmul(
                    rb[0:HD, :], lhsT=ones8r, rhs=sb, start=True, stop=True
                )
                rbs = epl.tile([HD, 512], F32, tag="rbs")
                nc.vector.reciprocal(rbs, rb[0:HD, :])
                prod = epl.tile([HD, 512], F32, tag="prod")
                nc.vector.scalar_tensor_tensor(
                    prod,
                    in0=acc[0:HD, :],
                    scalar=g8col[:],
                    in1=rbs,
                    op0=mybir.AluOpType.mult,
                    op1=mybir.AluOpType.mult,
                )
                fin = epl.tile([HD, 512], F32, tag="fin")
                nc.vector.tensor_add(fin, prod, xrs[h][:, j * 512 : (j + 1) * 512])
                nc.sync.dma_start(
                    out=out_d[och, j * 512 : (j + 1) * 512], in_=fin
                )


def build_bass():
    nc = bacc.Bacc("TRN2", target_bir_lowering=False, debug=False, num_devices=NCORES)
    xb_d = nc.dram_tensor("xb", [C, N], F32, kind="ExternalInput").ap()
    xres_d = nc.dram_tensor("xres", [CH, N], F32, kind="ExternalInput").ap()
    wcat_d = nc.dram_tensor("wcat", [C + 1, 3 * CH], F32, kind="ExternalInput").ap()
    gamma8_d = nc.dram_tensor("gamma8", [HD, 1], F32, kind="ExternalInput").ap()
    out_d = nc.dram_tensor("out", [CH, N], F32, kind="ExternalOutput").ap()
    scr = [
        nc.dram_tensor("scr0", [CH, N], F32R).ap(),
        nc.dram_tensor("scr1", [CH, N], F32R).ap(),
        nc.dram_tensor("scr2", [CH, N], BF16).ap(),
    ]

    with tile.TileContext(nc) as tc:
        _emit(tc, xb_d, xres_d, wcat_d, gamma8_d, out_d, scr)
    nc.finalize()
    return nc


_NC = None


def _get_nc():
    global _NC
    if _NC is None:
        _NC = build_bass()
    return _NC


def make_in_maps(x, wq, bq, wk, bk, wv, bv, gamma):
    x = np.asarray(x, np.float32)
    gamma = np.asarray(gamma, np.float32).reshape(-1)
    in_maps = []
    for core in range(NCORES):
        b, p = divmod(core, 4)
        sl = slice(CH * p, CH * (p + 1))
        wcat = np.empty((C + 1, 3 * CH), np.float32)
        for t, (w, bias) in enumerate(((wq, bq), (wk, bk), (wv, bv))):
            wcat[:C, t * CH : (t + 1) * CH] = np.asarray(w, np.float32)[sl].T
            wcat[C, t * CH : (t + 1) * CH] = np.asarray(bias, np.float32)[sl]
        in_maps.append(
            {
                "xb": np.ascontiguousarray(x[b].reshape(C, N)),
                "xres": np.ascontiguousarray(x[b, sl].reshape(CH, N)),
                "wcat": wcat,
                "gamma8": np.full((HD, 1), gamma[0], np.float32),
            }
        )
    return in_maps


def assemble_out(results):
    out = np.empty((B, C, N), np.float32)
    for core in range(NCORES):
        b, p = divmod(core, 4)
        out[b, CH * p : CH * (p + 1)] = results[core]["out"]
    return out.reshape(B, C, 64, 64)


def kernel(x, wq, bq, wk, bk, wv, bv, gamma):
    nc = _get_nc()
    in_maps = make_in_maps(x, wq, bq, wk, bk, wv, bv, gamma)
    res = run_bass_kernel_spmd(nc, in_maps, list(range(NCORES))).results
    return assemble_out(res)


if __name__ == "__main__":
    rng = np.random.default_rng(0)
    x = rng.standard_normal((B, C, 64, 64), dtype=np.float32)
    wq, wk, wv = (
        rng.standard_normal((C, C), dtype=np.float32) / 8.0 for _ in range(3)
    )
    bq, bk, bv = (
        rng.standard_normal((C,), dtype=np.float32) * 0.01 for _ in range(3)
    )
    gamma = rng.random((1,), dtype=np.float32)
    out = kernel(x, wq, bq, wk, bk, wv, bv, gamma)
    print(out.shape, out.dtype)



# revision 2
# speedup vs baseline: 2.3411x; 2.3411x over previous
"""Multi-head attention (B=2, C=64, H=W=64, nh=8) on 8 TRN2 NeuronCores.

Sharding: 16 (batch, head) pairs -> 2 consecutive heads per core.
core = 4*b + p handles batch b, heads {2p, 2p+1} = conv channels [16p, 16p+16).

Dispatch architecture (the wall clock here is dominated by the axon tunnel:
~80ms RTT floor per synchronous drain, ~50MB/s transfer):
  - host uploads x ONCE (2MB, rides the RTT window) sharded 16 rows/core;
    a jax prep jit all_gathers it on-device over NeuronLink and slices each
    core's full x[b] (conv needs all 64 input channels) + its 16 residual
    rows -- instead of uploading the 4x-duplicated 8MB from the host.
  - the bass kernel's zero output operands live on device permanently
    (not donated, never re-uploaded).
  - output is fp16 (1MB instead of 2MB D2H), upcast host-side.

Per-core bass pipeline (all on-chip, scores never hit HBM):
  conv1x1 (PE fp32r, bias via ones-row)
  -> DRAM-bounce gathers to build Q^T/K^T [8,4096] (fp32r) and V-chunked
     [128,32,33] (bf16, ones column for the softmax denominator)
     (torch .view semantics: Q[n,d] = conv[ch0 + n//512, (n%512)*8 + d])
  -> S^T tiles [128m, 512n] = K Q^T on PE (fp32r), exp on ACT with
     1/sqrt(8) folded into the activation scale, bf16 output
     (no max subtraction; scores are O(1) by construction)
  -> PV via augmented V|ones bf16 matmul -> [33, 512] psum (row 32 = denom)
  -> normalize: broadcast denom via PE ones outer product, DVE reciprocal,
     fused gamma*pv*recip (scalar_tensor_tensor), add residual, fp16 out.
"""

import numpy as np

import concourse.bacc as bacc
import concourse.bass as bass
import concourse.tile as tile
from concourse import mybir

F32 = mybir.dt.float32
F32R = mybir.dt.float32r
BF16 = mybir.dt.bfloat16
F16 = mybir.dt.float16

B = 2
C = 64
N = 4096          # H*W
NH = 8
HD = 8            # head dim
HPC = 2           # heads per core
CH = HPC * HD     # 16 conv channels per core
NCORES = 8
NBLK = N // 512   # 8 n-blocks of 512 query positions
MCHUNK = N // 128  # 32 m-chunks of 128 key positions
EXPW = 1536       # elements exp'd per ACT instruction (psum banks = EXPW/512)
ST_BUFS = 2       # st tile double buffering
ACC_BUFS = 2      # PV accumulator buffering
SCALE = 1.0 / np.sqrt(float(HD))


def _chunk_groups():
    """Partition the 32 m-chunks into groups of <= EXPW//512 for one exp each."""
    per = EXPW // 512
    groups, k = [], 0
    while k < MCHUNK:
        n = min(per, MCHUNK - k)
        groups.append(list(range(k, k + n)))
        k += n
    return groups


def _emit(tc, xb_d, xres_d, wcat_d, gamma8_d, out_d, scr):
    nc = tc.nc

    with (
        tc.tile_pool(name="persist", bufs=1) as per,
        tc.tile_pool(name="ptp", bufs=3) as ptp,
        tc.tile_pool(name="epl", bufs=3) as epl,
        tc.tile_pool(name="stp", bufs=ST_BUFS, space="PSUM") as stp,
        tc.tile_pool(name="accp", bufs=ACC_BUFS, space="PSUM") as accp,
    ):
        # ---- persistent tiles ----
        g8col = per.tile([HD, 1], F32)            # gamma per-partition scalar
        ones8r = per.tile([1, HD], F32R)          # broadcast lhsT
        xrs = [per.tile([HD, N], F32, name=f"xrs{h}", tag=f"xrs{h}") for h in range(HPC)]
        qt = [per.tile([HD, N], F32R, name=f"qt{h}", tag=f"qt{h}") for h in range(HPC)]
        kt = [per.tile([HD, N], F32R, name=f"kt{h}", tag=f"kt{h}") for h in range(HPC)]
        vc = [per.tile([128, MCHUNK, 33], BF16, name=f"vc{h}", tag=f"vc{h}")
              for h in range(HPC)]

        # ---- loads + fp32r casts ----
        nc.sync.dma_start(out=g8col[:], in_=gamma8_d[:])
        for h in range(HPC):
            nc.sync.dma_start(out=xrs[h][:], in_=xres_d[h * HD : (h + 1) * HD, :])

        o8f = per.tile([1, HD], F32)
        nc.vector.memset(o8f, 1.0)
        nc.vector.tensor_copy(ones8r, o8f)        # rounds to fp32r

        with tc.tile_pool(name="convin", bufs=1) as cin:
            xba = cin.tile([C + 1, N], F32)       # x[b] + ones row (bias)
            xbar = cin.tile([C + 1, N], F32R)
            nc.vector.memset(xba[C : C + 1, :], 1.0)
            nc.sync.dma_start(out=xba[0:C, :], in_=xb_d[:])
            nc.vector.tensor_copy(xbar, xba)      # rounds to fp32r

            wcf = cin.tile([C + 1, 3 * CH], F32)
            nc.sync.dma_start(out=wcf[:], in_=wcat_d[:])
            wcr = cin.tile([C + 1, 3 * CH], F32R)
            nc.vector.tensor_copy(wcr, wcf)       # rounds to fp32r

            # ---- conv1x1: [16,4096] = wT.T @ [65,4096] per q/k/v ----
            with tc.tile_pool(name="convout", bufs=1) as cop:
                cqkv = [
                    cop.tile([CH, N], F32R if t < 2 else BF16, name=f"c{t}", tag=f"c{t}")
                    for t in range(3)
                ]
                for t in range(3):
                    lhsT = wcr[:, t * CH : (t + 1) * CH]
                    for j in range(NBLK):
                        ps = stp.tile([128, EXPW], F32, tag="st")
                        nc.tensor.matmul(
                            ps[0:CH, 0:512],
                            lhsT=lhsT,
                            rhs=xbar[:, j * 512 : (j + 1) * 512],
                            start=True,
                            stop=True,
                        )
                        nc.vector.tensor_copy(
                            cqkv[t][:, j * 512 : (j + 1) * 512], ps[0:CH, 0:512]
                        )
                    nc.sync.dma_start(out=scr[t][:], in_=cqkv[t][:])


                # ---- re-layout gathers (torch .view semantics) ----
                for h in range(HPC):
                    rows = slice(h * HD, (h + 1) * HD)
                    # Q^T/K^T [d, n]: element = conv[row r, col 8t+d], n = 512r+t
                    for r0 in range(0, HD, 2):
                        nc.sync.dma_start(
                            out=qt[h][:].rearrange("d (r t) -> d r t", r=HD)[
                                :, r0 : r0 + 2, :
                            ],
                            in_=scr[0][rows, :].rearrange("r (t d) -> d r t", d=HD)[
                                :, r0 : r0 + 2, :
                            ],
                        )
                        nc.sync.dma_start(
                            out=kt[h][:].rearrange("d (r t) -> d r t", r=HD)[
                                :, r0 : r0 + 2, :
                            ],
                            in_=scr[1][rows, :].rearrange("r (t d) -> d r t", d=HD)[
                                :, r0 : r0 + 2, :
                            ],
                        )
                    # V chunked [i, chunk, d]: m = 128*chunk + i, chunk = 4r+tb
                    nc.sync.dma_start(
                        out=vc[h][:, :, 0:HD],
                        in_=scr[2][rows, :].rearrange(
                            "r (tb i d) -> i (r tb) d", tb=4, i=128, d=HD
                        ),
                    )
                    nc.vector.memset(vc[h][:, :, HD:32], 0.0)
                    nc.vector.memset(vc[h][:, :, 32:33], 1.0)

        # ---- attention per head / n-block ----
        for h in range(HPC):
            och = slice(h * HD, (h + 1) * HD)
            for j in range(NBLK):
                qblk = qt[h][:, j * 512 : (j + 1) * 512]
                acc = accp.tile([33, 512], F32, tag="acc")
                for grp in _chunk_groups():
                    st = stp.tile([128, EXPW], F32, tag="st")
                    for u, k in enumerate(grp):
                        nc.tensor.matmul(
                            st[:, u * 512 : (u + 1) * 512],
                            lhsT=kt[h][:, k * 128 : (k + 1) * 128],
                            rhs=qblk,
                            start=True,
                            stop=True,
                        )
                    w = len(grp) * 512
                    pt = ptp.tile([128, EXPW], BF16)
                    nc.scalar.activation(
                        pt[:, 0:w], st[:, 0:w],
                        mybir.ActivationFunctionType.Exp, scale=SCALE
                    )
                    for u, k in enumerate(grp):
                        nc.tensor.matmul(
                            acc[:, :],
                            lhsT=vc[h][:, k, :],
                            rhs=pt[:, u * 512 : (u + 1) * 512],
                            start=(k == 0),
                            stop=(k == MCHUNK - 1),
                        )

                # ---- epilogue: out = gamma * pv / denom + residual ----
                sb = epl.tile([1, 512], F32R, tag="sb")
                nc.vector.tensor_copy(sb, acc[32:33, :])  # denom -> fp32r
                rb = accp.tile([33, 512], F32, tag="acc")
                nc.tensor.matmul(
                    rb[0:HD, :], lhsT=ones8r, rhs=sb, start=True, stop=True
                )
                rbs = epl.tile([HD, 512], F32, tag="rbs")
                nc.vector.reciprocal(rbs, rb[0:HD, :])
                prod = epl.tile([HD, 512], F32, tag="prod")
                nc.vector.scalar_tensor_tensor(
                    prod,
                    in0=acc[0:HD, :],
                    scalar=g8col[:],
                    in1=rbs,
                    op0=mybir.AluOpType.mult,
                    op1=mybir.AluOpType.mult,
                )
                fin = epl.tile([HD, 512], F16, tag="fin")
                nc.vector.tensor_add(fin, prod, xrs[h][:, j * 512 : (j + 1) * 512])
                nc.sync.dma_start(
                    out=out_d[och, j * 512 : (j + 1) * 512], in_=fin
                )


def build_bass():
    nc = bacc.Bacc("TRN2", target_bir_lowering=False, debug=False, num_devices=NCORES)
    xb_d = nc.dram_tensor("xb", [C, N], F32, kind="ExternalInput").ap()
    xres_d = nc.dram_tensor("xres", [CH, N], F32, kind="ExternalInput").ap()
    wcat_d = nc.dram_tensor("wcat", [C + 1, 3 * CH], F32, kind="ExternalInput").ap()
    gamma8_d = nc.dram_tensor("gamma8", [HD, 1], F32, kind="ExternalInput").ap()
    out_d = nc.dram_tensor("out", [CH, N], F16, kind="ExternalOutput").ap()
    scr = [
        nc.dram_tensor("scr0", [CH, N], F32R).ap(),
        nc.dram_tensor("scr1", [CH, N], F32R).ap(),
        nc.dram_tensor("scr2", [CH, N], BF16).ap(),
    ]

    with tile.TileContext(nc) as tc:
        _emit(tc, xb_d, xres_d, wcat_d, gamma8_d, out_d, scr)
    nc.finalize()
    return nc


# ---------------- host / dispatch side ----------------

_STATE = None


def _build_state():
    import jax
    from jax.sharding import Mesh, PartitionSpec, NamedSharding
    from jax.experimental.shard_map import shard_map
    from concourse import bass2jax

    nc = build_bass()
    bass2jax.install_neuronx_cc_hook()

    partition_name = nc.partition_id_tensor.name if nc.partition_id_tensor else None
    in_names, out_names, out_avals = [], [], []
    for alloc in nc.m.functions[0].allocations:
        if not isinstance(alloc, mybir.MemoryLocationSet):
            continue
        name = alloc.memorylocations[0].name
        if alloc.kind == "ExternalInput":
            if name != partition_name:
                in_names.append(name)
        elif alloc.kind == "ExternalOutput":
            out_names.append(name)
            out_avals.append(
                jax.core.ShapedArray(tuple(alloc.tensor_shape), mybir.dt.np(alloc.dtype))
            )
    n_params = len(in_names)
    all_names = list(in_names) + out_names
    bind_names = list(all_names)
    if partition_name is not None:
        bind_names.append(partition_name)

    devices = jax.devices()[:NCORES]
    assert len(devices) == NCORES, f"need {NCORES} devices, got {len(jax.devices())}"
    mesh = Mesh(np.asarray(devices), ("core",))
    P = PartitionSpec
    csh = NamedSharding(mesh, P("core"))

    # --- prep jit: upload x once, all_gather on-device, slice per-core views ---
    def _prep_body(xs):
        g = jax.lax.all_gather(xs, "core", axis=0, tiled=True)   # [128, N] full x
        c = jax.lax.axis_index("core")
        b = c // 4
        p = c - 4 * b
        xb = jax.lax.dynamic_slice(g, (C * b, 0), (C, N))
        xres = jax.lax.dynamic_slice(g, (C * b + CH * p, 0), (CH, N))
        return xb, xres

    prep = jax.jit(
        shard_map(_prep_body, mesh=mesh, in_specs=P("core"),
                  out_specs=(P("core"), P("core")), check_rep=False)
    )

    # --- bass exec jit (no donation: zero out-operands live on device forever) ---
    def _bass_body(*args):
        operands = list(args)
        if partition_name is not None:
            operands.append(bass2jax.partition_id_tensor())
        outs = bass2jax._bass_exec_p.bind(
            *operands,
            out_avals=tuple(out_avals),
            in_names=tuple(bind_names),
            out_names=tuple(out_names),
            lowering_input_output_aliases=(),
            sim_require_finite=True,
            sim_require_nnan=True,
            nc=nc,
        )
        return tuple(outs)

    run = jax.jit(
        shard_map(_bass_body, mesh=mesh,
                  in_specs=(P("core"),) * (n_params + len(out_names)),
                  out_specs=(P("core"),) * len(out_names),
                  check_rep=False),
        keep_unused=True,
    )

    zeros = [
        jax.device_put(
            np.zeros((NCORES * av.shape[0], *av.shape[1:]), av.dtype), csh
        )
        for av in out_avals
    ]

    return {
        "prep": prep,
        "run": run,
        "zeros": zeros,
        "in_names": in_names,
        "csh": csh,
    }


def _get_state():
    global _STATE
    if _STATE is None:
        _STATE = _build_state()
    return _STATE


def _host_weights(wq, bq, wk, bk, wv, bv, gamma):
    wcat = np.empty((NCORES, C + 1, 3 * CH), np.float32)
    for p in range(4):
        sl = slice(CH * p, CH * (p + 1))
        for t, (w, bias) in enumerate(((wq, bq), (wk, bk), (wv, bv))):
            wcat[p, :C, t * CH : (t + 1) * CH] = np.asarray(w, np.float32)[sl].T
            wcat[p, C, t * CH : (t + 1) * CH] = np.asarray(bias, np.float32)[sl]
    wcat[4:] = wcat[:4]          # cores 4..7: same head slice, other batch
    gamma = np.asarray(gamma, np.float32).reshape(-1)
    gamma_all = np.full((NCORES * HD, 1), gamma[0], np.float32)
    return wcat.reshape(NCORES * (C + 1), 3 * CH), gamma_all


def kernel(x, wq, bq, wk, bk, wv, bv, gamma):
    st = _get_state()
    x = np.asarray(x, np.float32)
    xg = x.reshape(B * C, N)
    if not xg.flags.c_contiguous:
        xg = np.ascontiguousarray(xg)
    xb_g, xres_g = st["prep"](xg)
    wcat_all, gamma_all = _host_weights(wq, bq, wk, bk, wv, bv, gamma)
    arrs = {"xb": xb_g, "xres": xres_g, "wcat": wcat_all, "gamma8": gamma_all}
    outs = st["run"](*[arrs[n] for n in st["in_names"]], *st["zeros"])
    out = np.asarray(outs[0]).astype(np.float32)
    return out.reshape(B, C, 64, 64)


if __name__ == "__main__":
    rng = np.random.default_rng(0)
    x = rng.standard_normal((B, C, 64, 64), dtype=np.float32)
    wq, wk, wv = (
        rng.standard_normal((C, C), dtype=np.float32) / 8.0 for _ in range(3)
    )
    bq, bk, bv = (
        rng.standard_normal((C,), dtype=np.float32) * 0.01 for _ in range(3)
    )
    gamma = rng.random((1,), dtype=np.float32)
    out = kernel(x, wq, bq, wk, bk, wv, bv, gamma)
    print(out.shape, out.dtype)
